# revision 1
# baseline (speedup 1.0000x reference)
"""VQ codebook cosine-similarity softmax kernel for Trainium2 (8 NeuronCores).

Computes softmax(cos_sim(batch, centroids)) for batch [131072, 1024] f32 and
centroids [256, 1024] f32, data-parallel over the batch dim across 8 cores.

Per-core pipeline (16384 rows):
  - SWDGE cast-DMA loads x tiles HBM f32 -> SBUF fp16 (halves SBUF traffic,
    enables full-rate fp16 matmuls; fp32 PSUM accumulation keeps rel err ~5e-5)
  - PE transposes each [128,128] fp16 block (x needs D on partitions for the
    matmul contraction); PSUM->SBUF copyback split between DVE and ACT
  - PE matmul: weights = xT block [128d,128n], moving = cnT [128d,256k],
    accumulating over 8 d-chunks into PSUM f32 [128n, 256k]
  - row norms on DVE: tensor_tensor_reduce(x*x) then rsqrt via the
    0x5f3759df bit trick + 3 Newton steps (keeps Ln/Sqrt off ACT so only
    the Exp table set ever loads)
  - softmax: logits = cos in [-1,1] so no max-subtraction needed;
    ACT Exp(scale=1/||x||) with accum_out giving the denominator,
    DVE reciprocal + ACT Copy(scale=1/denom) for the final normalize
"""

import os
import sys

if "/opt/trn_rl_repo" not in sys.path:
    sys.path.insert(0, "/opt/trn_rl_repo")

import numpy as np

RSQRT_MODE = os.environ.get("KM_RSQRT_MODE", "bit")  # bit | act
MUL_MODE = os.environ.get("KM_MUL_MODE", "act")  # act | dve
COPY_SPLIT = os.environ.get("KM_COPY_SPLIT", "0") == "1"  # DVE+ACT vs DVE only
SQ_MODE = os.environ.get("KM_SQ_MODE", "sts")  # sts | ttr | act
# NOTE: ttr (tensor_tensor_reduce) compiles and simulates fine but faults the
# device at runtime — do not use. sts (scalar_tensor_tensor + accum) works.
# SQ_SPLIT: columns of each row handled by DVE (sts); the rest go to ACT
# (Square). Both run at 1 elem/cycle/lane, so this splits the norm pass
# across the two engines. 0 = all ACT, 1024 = all DVE.
SQ_SPLIT = int(os.environ.get("KM_SQ_SPLIT", "640"))
EARLY_CLOSE = os.environ.get("KM_EARLY_CLOSE", "1") == "1"
SPS_BUFS = int(os.environ.get("KM_SPS_BUFS", "4"))
XT_BUFS = int(os.environ.get("KM_XT_BUFS", "4"))
E_BUFS = int(os.environ.get("KM_E_BUFS", "6"))
DEN_BUFS = int(os.environ.get("KM_DEN_BUFS", "6"))
NRM_BUFS = int(os.environ.get("KM_NRM_BUFS", "4"))


N, D, K = 131072, 1024, 256
NCORES = 8
NPC = N // NCORES  # rows per core
P = 128  # partitions / tile rows
XB = 4  # row-tiles per load/store DMA batch
G = 16  # row-tiles per norm group (batched rsqrt)
F1 = 832  # copyback columns done by DVE (rest by ACT)

RSQRT_MAGIC = 0x5F3759DF


def build_bass(npc=NPC):
    """Build the single-core SPMD program; every core runs this with its own
    x shard. Returns the compiled Bacc object."""
    from contextlib import ExitStack

    import concourse.bacc as bacc
    import concourse.mybir as mybir
    import concourse.tile as tile
    from concourse.masks import make_identity

    dt = mybir.dt
    AFT = mybir.ActivationFunctionType
    Alu = mybir.AluOpType

    nt = npc // P  # row tiles
    assert npc % (P * XB) == 0
    ngroups = (nt + G - 1) // G

    nc = bacc.Bacc(
        "TRN2", target_bir_lowering=False, debug=False, num_devices=NCORES
    )
    x_d = nc.dram_tensor("x", [npc, D], dt.float32, kind="ExternalInput")
    c_d = nc.dram_tensor("c", [K, D], dt.float32, kind="ExternalInput")
    o_d = nc.dram_tensor("o", [npc, K], dt.float32, kind="ExternalOutput")

    ND = D // P  # d-chunks (8)

    def emit_rsqrt(nc, dst, src, scratch_a, scratch_b, w):
        """dst[:, :w] = 1/sqrt(src[:, :w]).

        bit mode: 0x5f3759df bit trick + 3 Newton steps, all on DVE.
        act mode: exp(-0.5*ln(src)) seed on ACT + 1 Newton step on DVE.
        """
        if RSQRT_MODE == "bit":
            srci = src.bitcast(dt.int32)
            dsti = dst.bitcast(dt.int32)
            nc.vector.tensor_scalar(
                dsti, srci, 1, None, Alu.logical_shift_right
            )
            # magic - x == (x ^ 0xffffffff) + (magic + 1)  (avoids int negate)
            nc.vector.tensor_scalar(dsti, dsti, -1, None, Alu.bitwise_xor)
            nc.vector.tensor_scalar(dsti, dsti, RSQRT_MAGIC + 1, None, Alu.add)
            niter = 3
        else:
            nc.scalar.activation(scratch_a, src, AFT.Ln)
            nc.scalar.activation(dst, scratch_a, AFT.Exp, scale=-0.5)
            niter = 1
        for _ in range(niter):
            nc.vector.tensor_tensor(scratch_a, dst, dst, Alu.mult)
            nc.vector.tensor_tensor(scratch_b, scratch_a, src, Alu.mult)
            nc.vector.tensor_scalar(
                scratch_b, scratch_b, -0.5, 1.5, Alu.mult, Alu.add
            )
            nc.vector.tensor_tensor(dst, dst, scratch_b, Alu.mult)

    with tile.TileContext(nc) as tc, ExitStack() as ctx:
        const = ctx.enter_context(tc.tile_pool(name="const", bufs=1))
        ident = const.tile([P, P], dt.float16)
        make_identity(nc, ident[:])

        # cnT: [128 (d within chunk), ND * K] fp16; chunk b at cols [K*b, K*b+K)
        cnT = const.tile([P, ND * K], dt.float16)
        # per-tile squared row norms (partial sums: a=DVE part, b=ACT part)
        n2a = const.tile([P, max(nt, 1)], dt.float32)
        n2b = const.tile([P, max(nt, 1)], dt.float32)
        # per-tile softmax denominators and their reciprocals
        denscols = const.tile([P, max(nt, 1)], dt.float32)
        rdenscols = const.tile([P, max(nt, 1)], dt.float32)

        # ---- centroid prep (one-time, ~1MB); pools close before main loop ----
        with ExitStack() as _cstack:
            cctx = _cstack if EARLY_CLOSE else ctx
            cprep = cctx.enter_context(tc.tile_pool(name="cprep", bufs=2))
            cpsum = cctx.enter_context(
                tc.tile_pool(name="cpsum", bufs=2, space="PSUM")
            )
            for h in range(K // P):  # 2 halves of the K=256 centroids
                c32 = cprep.tile([P, D], dt.float32, tag="c32")
                nc.sync.dma_start(c32[:], c_d.ap()[P * h : P * (h + 1), :])
                csq = cprep.tile([P, D], dt.float32, tag="csq")
                cn2 = cprep.tile([P, 1], dt.float32, tag="cn2")
                if SQ_MODE == "ttr":
                    nc.vector.tensor_tensor_reduce(
                        csq[:], c32[:], c32[:], 1.0, 0.0, Alu.mult, Alu.add,
                        accum_out=cn2[:],
                    )
                elif SQ_MODE == "sts":
                    nc.vector.scalar_tensor_tensor(
                        csq[:], c32[:], 1.0, c32[:], Alu.mult, Alu.mult,
                        accum_out=cn2[:],
                    )
                else:
                    nc.scalar.activation(
                        csq[:], c32[:], AFT.Square, accum_out=cn2[:]
                    )
                crn = cprep.tile([P, 1], dt.float32, tag="crn")
                csa = cprep.tile([P, 1], dt.float32, tag="csa")
                csb = cprep.tile([P, 1], dt.float32, tag="csb")
                emit_rsqrt(nc, crn[:], cn2[:], csa[:], csb[:], 1)
                cn16 = cprep.tile([P, D], dt.float16, tag="cn16")
                nc.vector.tensor_scalar_mul(cn16[:], c32[:], crn[:])
                for b in range(ND):
                    pt = cpsum.tile([P, P], dt.float16, tag="ct_ps")
                    nc.tensor.transpose(
                        pt[:], cn16[:, P * b : P * (b + 1)], ident[:]
                    )
                    nc.vector.tensor_copy(
                        cnT[:, K * b + P * h : K * b + P * h + P], pt[:]
                    )

        # ---- main loop ----
        x16_pool = ctx.enter_context(tc.tile_pool(name="x16", bufs=2 * G // XB))
        xt_pool = ctx.enter_context(tc.tile_pool(name="xt", bufs=XT_BUFS))
        sq_pool = ctx.enter_context(tc.tile_pool(name="sq", bufs=2))
        e_pool = ctx.enter_context(tc.tile_pool(name="e", bufs=E_BUFS))
        pm_pool = ctx.enter_context(tc.tile_pool(name="pm", bufs=3))
        nrm_pool = ctx.enter_context(tc.tile_pool(name="nrm", bufs=NRM_BUFS))
        den_pool = ctx.enter_context(tc.tile_pool(name="den", bufs=DEN_BUFS))
        tps_pool = ctx.enter_context(
            tc.tile_pool(name="tps", bufs=2, space="PSUM")
        )
        sps_pool = ctx.enter_context(
            tc.tile_pool(name="sps", bufs=SPS_BUFS, space="PSUM")
        )

        for g in range(ngroups):
            t0 = g * G
            t1 = min(t0 + G, nt)
            gtiles = range(t0, t1)
            gw = t1 - t0
            # 1) cast-loads (XB row-tiles per DMA)
            xmacs = {}
            for tm in range(t0 // XB, (t1 + XB - 1) // XB):
                xm = x16_pool.tile([P, XB * D], dt.float16, tag="xm")
                src = x_d.ap()[P * XB * tm : P * XB * (tm + 1), :].rearrange(
                    "(s p) d -> p s d", s=XB
                )
                nc.gpsimd.dma_start(
                    xm[:].rearrange("p (s d) -> p s d", s=XB), src
                )
                xmacs[tm] = xm
            # 2) row norms^2, split column-wise across DVE (sts) and ACT (Square)
            sd = max(0, min(D, SQ_SPLIT))
            for t in gtiles:
                xm = xmacs[t // XB]
                xs = xm[:, D * (t % XB) : D * (t % XB + 1)]
                if sd > 0:
                    sqa = sq_pool.tile([P, D], dt.float16, tag="sqa")
                    nc.vector.scalar_tensor_tensor(
                        sqa[:, :sd], xs[:, :sd], 1.0, xs[:, :sd],
                        Alu.mult, Alu.mult, accum_out=n2a[:, t : t + 1],
                    )
                if sd < D:
                    sqb = sq_pool.tile([P, D], dt.float16, tag="sqb")
                    nc.scalar.activation(
                        sqb[:, sd:], xs[:, sd:], AFT.Square,
                        accum_out=n2b[:, t : t + 1],
                    )
            # 3) batched rsqrt for the group's norms
            rng = nrm_pool.tile([P, G], dt.float32, tag="rng")
            nsa = nrm_pool.tile([P, G], dt.float32, tag="nsa")
            nsb = nrm_pool.tile([P, G], dt.float32, tag="nsb")
            n2s = nrm_pool.tile([P, G], dt.float32, tag="n2s")
            if sd == 0:
                n2src = n2b[:, t0:t1]
            elif sd == D:
                n2src = n2a[:, t0:t1]
            else:
                nc.vector.tensor_tensor(
                    n2s[:, :gw], n2a[:, t0:t1], n2b[:, t0:t1], Alu.add
                )
                n2src = n2s[:, :gw]
            emit_rsqrt(nc, rng[:, :gw], n2src, nsa[:, :gw], nsb[:, :gw], gw)
            # 4) per XB-block: transpose -> matmul -> exp, then batched
            #    reciprocal of the denominators, normalize, store
            for tm in range(t0 // XB, (t1 + XB - 1) // XB):
                bt0 = max(t0, tm * XB)
                bt1 = min(t1, (tm + 1) * XB)
                for t in range(bt0, bt1):
                    xm = xmacs[t // XB]
                    xs = xm[:, D * (t % XB) : D * (t % XB + 1)]
                    tps = tps_pool.tile([P, D], dt.float16, tag="tps")
                    for b in range(ND):
                        nc.tensor.transpose(
                            tps[:, P * b : P * (b + 1)],
                            xs[:, P * b : P * (b + 1)],
                            ident[:],
                        )
                    xt = xt_pool.tile([P, D], dt.float16, tag="xt")
                    if COPY_SPLIT:
                        nc.vector.tensor_copy(xt[:, :F1], tps[:, :F1])
                        nc.scalar.copy(xt[:, F1:], tps[:, F1:])
                    else:
                        nc.vector.tensor_copy(xt[:], tps[:])
                    sps = sps_pool.tile([P, K], dt.float32, tag="sps")
                    for b in range(ND):
                        nc.tensor.matmul(
                            sps[:],
                            xt[:, P * b : P * (b + 1)],
                            cnT[:, K * b : K * (b + 1)],
                            start=(b == 0),
                            stop=(b == ND - 1),
                        )
                    e = e_pool.tile([P, K], dt.float32, tag="e")
                    den = den_pool.tile([P, 1], dt.float32, tag="den")
                    j = t - t0
                    nc.scalar.activation(
                        e[:], sps[:], AFT.Exp,
                        scale=rng[:, j : j + 1], accum_out=den[:],
                    )
                    rden = den_pool.tile([P, 1], dt.float32, tag="rden")
                    nc.vector.reciprocal(rden[:], den[:])
                    if t == bt0:
                        pm = pm_pool.tile([P, XB * K], dt.float32, tag="pmac")
                    if MUL_MODE == "act":
                        nc.scalar.activation(
                            pm[:, K * (t % XB) : K * (t % XB + 1)],
                            e[:], AFT.Copy, scale=rden[:],
                        )
                    else:
                        nc.vector.tensor_scalar_mul(
                            pm[:, K * (t % XB) : K * (t % XB + 1)],
                            e[:], rden[:],
                        )
                dst = o_d.ap()[
                    P * XB * tm : P * XB * (tm + 1), :
                ].rearrange("(s p) k -> p s k", s=XB)
                nc.sync.dma_start(
                    dst, pm[:].rearrange("p (s k) -> p s k", s=XB)
                )

    nc.compile()
    return nc


_cache = {}


def _get_nc(npc=NPC):
    if npc not in _cache:
        _cache[npc] = build_bass(npc)
    return _cache[npc]


def kernel(batch: np.ndarray, centroids: np.ndarray) -> np.ndarray:
    from concourse.bass_utils import run_bass_kernel_spmd

    assert batch.shape == (N, D) and centroids.shape == (K, D)
    batch = np.ascontiguousarray(batch, dtype=np.float32)
    centroids = np.ascontiguousarray(centroids, dtype=np.float32)

    nc = _get_nc()
    in_maps = [
        {"x": batch[i * NPC : (i + 1) * NPC], "c": centroids}
        for i in range(NCORES)
    ]
    res = run_bass_kernel_spmd(nc, in_maps, core_ids=list(range(NCORES)))
    return np.concatenate([res.results[i]["o"] for i in range(NCORES)], axis=0)

